# revision 55
# baseline (speedup 1.0000x reference)
"""Trainium2 Bass kernel for MinibatchDiscrimination.

Computes out = concat([x, F], axis=1) where
  act = einsum('bd,kdm->bkm', x, W)               (B, K, M)
  D[b,k,c] = sum_m |act[b,k,m] - act[c,k,m]|      (B, K, B)
  F[b,k] = sum_c exp(-D[b,k,c])                   (B, K)

Sharding: tensor-parallel over nb_kernels (K=32) -> 4 kernels per core on 8
cores. Each core computes its F slice (B, 4); host concatenates [x | F].

Per-core algorithm:
  * actT[(k,m), b] = W_slice^T x^T via PE (bf16 in, f32 accum).
  * |x| = 2 relu(x) - x turns the pairwise L1 into a relu tensor_scalar
    (DVE, bf16 4x) + PE one-hot contraction over m, plus a rank-1 term
    rs[b] - rs[c] (rs = sum over relu-m of act) handled in factored form:
      exp(-D[b,c]) = w[b] * exp(-2 R[b,c]) * u[c].
    The diagonal stays exact: R[b,b] = 0 so exp(-2R) = 1 in bf16, and
    w[b]*u[b] = 1 +- 1e-7 in f32.
  * T-production slots (j-plane x 64-row group G) are split across DVE
    (relu), ACT (|x|/2 via Abs; those m's drop out of rs -> per-G rs
    variants A/B), and optionally GPSIMD (relu).
  * G0/G1 64-row matmuls are issued as interleaved pairs so they land in
    disjoint PE column groups (tile_position auto-derived) and run
    concurrently.
  * D is symmetric: only upper-triangle slab blocks are computed. Row
    parts come from the exp's accum_out; mirrored column parts are PE
    column sums of E accumulated across slabs in one PSUM row, copied to
    SBUF once per kernel and DMA-accumulated straight into f_out.
"""

import sys

if "/opt/trn_rl_repo" not in sys.path:
    sys.path.insert(0, "/opt/trn_rl_repo")

import numpy as np
import ml_dtypes

NB_KERNELS = 32
KERNEL_DIM = 16  # M
INPUT_DIM = 1024  # D
BATCH = 1024  # B
N_CORES = 8
KPC = NB_KERNELS // N_CORES  # kernels per core = 4

# slot (j, G) -> producing engine.  ACT slots use |x|/2 (Abs), so their
# m's are excluded from the rank-1 rs sums of that G's rows (variant A
# for G0, B for G1).  DVE/GPS slots are relu and do not affect rs.
J_ORDER = [0, 1, 6, 2, 7, 3, 5, 4]
GATHER_ORDER = [0, 5, 6, 7, 1, 2, 3, 4]  # ACT-plane scalars early for S_nh
ACT_JS = [5, 6, 7]  # planes that can run on ACT (need S_nh bias)
# rank-1 rs variants: per (G, slab-class) the relu-handled m's are m <
# RELU_M[v].  v0: G0 slabs 0-3 (ACT takes j5,6,7), v1: G0 slabs 4-7
# (ACT takes j6,7), v2: G1 all slabs (ACT takes j7).
RELU_M = [10, 12, 14]
NVAR = 3


def act_slot(j, G, slab):
    if (j, G) in {(6, 0), (7, 0), (7, 1)}:
        return True
    return j == 5 and G == 0 and slab < 4


def var_cls(G, slab):
    if G == 0:
        return 0 if slab < 4 else 1
    return 2

_CACHE = {}


def _build_program():
    import concourse.bass as bass
    import concourse.tile as tile
    from concourse import bacc, mybir

    bf16 = mybir.dt.bfloat16
    f32 = mybir.dt.float32
    Alu = mybir.AluOpType
    Act = mybir.ActivationFunctionType

    nc = bacc.Bacc(
        "TRN2",
        target_bir_lowering=False,
        debug=False,
        enable_asserts=False,
        num_devices=N_CORES,
    )

    KM = KPC * KERNEL_DIM  # 64 rows of actT
    NSLAB = BATCH // 128  # 8 slabs of b

    xT = nc.dram_tensor("xT", (INPUT_DIM, BATCH), bf16, kind="ExternalInput").ap()
    wT = nc.dram_tensor("wT", (INPUT_DIM, KM), bf16, kind="ExternalInput").ap()
    onehot = nc.dram_tensor("onehot", (128, 64), bf16, kind="ExternalInput").ap()
    # blockdiag[(k,m), v*KPC+k'] = 1 if k == k' and m < RELU_M[v]
    blockdiag = nc.dram_tensor(
        "blockdiag", (KM, NVAR * KPC), bf16, kind="ExternalInput"
    ).ap()
    # selk[v*KPC + k', (k*4 + G*2 + cls)*64 + b'] = 1 if k' == k and
    # v == var_cls(G, cls-slab-class)
    selk = nc.dram_tensor(
        "selk", (NVAR * KPC, KPC * 256), bf16, kind="ExternalInput"
    ).ap()
    f_out = nc.dram_tensor("f_out", (BATCH, KPC), f32, kind="ExternalOutput").ap()

    DCH = INPUT_DIM // 128  # 8 chunks of the matmul contraction dim

    def mm_chunks(span):
        off = 0
        while off < span:
            fd = min(512, span - off)
            yield off, fd
            off += fd

    with tile.TileContext(nc) as tc:
        with (
            tc.tile_pool(name="singles", bufs=1) as singles,
            tc.tile_pool(name="vk", bufs=2) as vk_pool,
            tc.tile_pool(name="sk", bufs=2) as sk_pool,
            tc.tile_pool(name="tg", bufs=12) as t_pool,
            tc.tile_pool(name="es", bufs=3) as e_pool,
            tc.tile_pool(name="fk", bufs=2) as fk_pool,
            tc.tile_pool(name="small", bufs=4) as small_pool,
            tc.tile_pool(name="dps", bufs=2, space="PSUM") as d_psum,
            tc.tile_pool(name="csps", bufs=1, space="PSUM") as cs_psum,
            tc.tile_pool(name="dram", bufs=1, space="DRAM") as dram_pool,
        ):
            # ---- Phase 1: actT = wT^T @ xT on PE; rs / u / w prep ----
            # Input loads spread over sync/scalar/vector queues so the
            # first matmul chunk can start ~2us in.
            wT_sb = singles.tile([128, DCH, KM], bf16)
            nc.sync.dma_start(out=wT_sb[:], in_=wT.rearrange("(i p) c -> p i c", p=128))
            xT_sb = singles.tile([128, DCH, BATCH], bf16)
            xT_r = xT.rearrange("(i p) b -> p i b", p=128)
            xT_q = [
                nc.sync, nc.scalar, nc.sync, nc.scalar,
                nc.sync, nc.scalar, nc.gpsimd, nc.gpsimd,
            ]
            for i in range(DCH):
                xT_q[i].dma_start(out=xT_sb[:, i, :], in_=xT_r[:, i, :])
            onehot_sb = singles.tile([128, 64], bf16)
            nc.sync.dma_start(out=onehot_sb[:], in_=onehot)
            blockdiag_sb = singles.tile([KM, NVAR * KPC], bf16)
            nc.sync.dma_start(out=blockdiag_sb[:], in_=blockdiag)
            selk_sb = singles.tile([NVAR * KPC, KPC * 256], bf16)
            nc.sync.dma_start(out=selk_sb[:], in_=selk)
            ones_sb = singles.tile([128, 1], bf16)
            nc.vector.memset(ones_sb[:], 1.0)

            # Phase-1 chain is split into 512-col halves so the copy /
            # cast / rs / bounce work of half 0 overlaps half 1's matmuls.
            actT_sb = singles.tile([KM, BATCH], bf16)
            actT_f32 = singles.tile([KM, BATCH], f32)
            actT_dram = dram_pool.tile([KM, BATCH], bf16)
            actT32_dram = dram_pool.tile([KM, BATCH], f32)
            act_ps = d_psum.tile([KM, BATCH], f32, tag="D")
            rs_ps = cs_psum.tile([NVAR * KPC, BATCH], f32, tag="rs", bufs=1)
            # bf16 rounding commutes with *2, so 2*rsh == rs2 exactly; the
            # exp argument on the diagonal cancels to 0.  Rows v*KPC..
            # v*KPC+3 hold variant v.
            rsh = singles.tile([NVAR * KPC, BATCH], bf16)  # bf16(-rs/2)
            rs2 = singles.tile([NVAR * KPC, BATCH], bf16)  # bf16(-rs)
            rs2_dram = dram_pool.tile([NVAR * KPC, BATCH], bf16)
            for h in range(BATCH // 512):
                hs = slice(h * 512, (h + 1) * 512)
                for i in range(DCH):
                    nc.tensor.matmul(
                        act_ps[:, hs],
                        lhsT=wT_sb[:, i, :],
                        rhs=xT_sb[:, i, hs],
                        start=(i == 0),
                        stop=(i == DCH - 1),
                    )
                nc.scalar.copy(actT_sb[:, hs], act_ps[:, hs])
                # f32 upconvert of the bf16-rounded actT (tensor_scalar
                # scalars must be f32 but must equal V's bf16 values).
                nc.vector.tensor_copy(actT_f32[:, hs], actT_sb[:, hs])
                # DRAM bounces: the broadcast/gather DMAs below need
                # arbitrary strided (incl. 0-step) source APs, which SBUF
                # sources disallow.
                nc.sync.dma_start(out=actT_dram[:, hs], in_=actT_sb[:, hs])
                nc.sync.dma_start(out=actT32_dram[:, hs], in_=actT_f32[:, hs])
            # rs variants over the *bf16* actT (so the 2relu(x)-x identity
            # is exact w.r.t. the values relu sees).  Kept off the sync
            # queue so the V/S gathers below aren't blocked behind it.
            for h in range(BATCH // 512):
                hs = slice(h * 512, (h + 1) * 512)
                nc.tensor.matmul(
                    rs_ps[:, hs],
                    lhsT=blockdiag_sb[:],
                    rhs=actT_sb[:, hs],
                    start=True,
                    stop=True,
                )
            nc.scalar.mul(rsh[:], rs_ps[:], -0.5)
            nc.scalar.mul(rs2[:], rs_ps[:], -1.0)
            nc.scalar.dma_start(out=rs2_dram[:], in_=rs2[:])

            # bias_cols[p, slab*KPC+k] = bf16(-rs[var_cls(G(p), slab)][k,
            # slab*128+p]) -- slab-major; per-(G, slab) gathers (the AP
            # balancer can't merge the 3-dim form).  Emitted later, off
            # the k=0 critical path.
            bias_cols = singles.tile([128, NSLAB * KPC], bf16)
            rc0 = rs2_dram[0:1, 0:1]

            def emit_bias_cols():
                for slab in range(NSLAB):
                    for G in range(2):
                        v = var_cls(G, slab)
                        nc.scalar.dma_start(
                            out=bias_cols[
                                64 * G : 64 * (G + 1),
                                slab * KPC : (slab + 1) * KPC,
                            ],
                            in_=bass.AP(
                                tensor=rc0.tensor,
                                offset=rc0.offset
                                + v * KPC * BATCH
                                + slab * 128
                                + G * 64,
                                ap=[[1, 64], [BATCH, KPC]],
                            ),
                        )

            # ---- per-kernel input staging ----
            # Partition layout of T tiles, per plane j in 0..7:
            #   partition p holds m = 2j + p//64 and b_loc = p % 64.
            # V[p, j, c] = actT_bf16[k*16 + 2j + p//64, c]
            # S[p, j, f] = actT_f32 [k*16 + 2j + p//64,
            #                        slab*128 + G*64 + p%64],  f = slab*2+G
            Vk = {}
            Sk = {}
            Snhk = {}

            def emit_inputs(k, v_eng, s_engs, js=None, with_v=True, order=None):
                base = k * KERNEL_DIM
                if k not in Vk:
                    Vk[k] = {}
                    Sk[k] = sk_pool.tile([128, 8, 16], f32, name=f"S{k}")
                Vp, S = Vk[k], Sk[k]
                if order is None:
                    order = [j for j in GATHER_ORDER if js is None or j in js]
                for idx, j in enumerate(order):
                    if with_v:
                        Vp[j] = vk_pool.tile(
                            [128, BATCH], bf16, name=f"V{k}_{j}", tag=f"v{j}"
                        )
                    for q in range(2):
                        row = base + 2 * j + q
                        if with_v:
                            row_b = actT_dram[row : row + 1, 0:1]
                            v_eng.dma_start(
                                out=Vp[j][64 * q : 64 * (q + 1), :],
                                in_=bass.AP(
                                    tensor=row_b.tensor,
                                    offset=row_b.offset,
                                    ap=[[0, 64], [1, BATCH]],
                                ),
                            )
                        row_s = actT32_dram[row : row + 1, 0:1]
                        s_eng = (
                            s_engs[idx % len(s_engs)]
                            if isinstance(s_engs, list)
                            else s_engs(j)
                        )
                        s_eng.dma_start(
                            out=S[64 * q : 64 * (q + 1), j, :],
                            in_=bass.AP(
                                tensor=row_s.tensor,
                                offset=row_s.offset,
                                ap=[[1, 64], [128, NSLAB], [64, 2]],
                            ),
                        )

            def emit_snh(k):
                # S_nh = -S/2: ACT-slot bias (Abs(0.5*V - 0.5*s) = |x|/2)
                S_nh = sk_pool.tile([128, len(ACT_JS), 16], f32, tag="snh")
                for i, j in enumerate(ACT_JS):
                    nc.vector.tensor_scalar(
                        out=S_nh[:, i, :],
                        in0=Sk[k][:, j, :],
                        scalar1=-0.5,
                        scalar2=None,
                        op0=Alu.mult,
                    )
                Snhk[k] = (S_nh, {j: i for i, j in enumerate(ACT_JS)})

            # k=0 inputs on fast HWDGE queues right after the bounces, in
            # consumption-priority order (S_nh needs j5/6/7 scalars early,
            # but j0/j1 tiles are consumed first).
            emit_inputs(
                0,
                nc.sync,
                lambda j: nc.scalar if j in (1, 2, 3, 4) else nc.sync,
                order=[0, 1, 6, 7, 5, 2, 3, 4],
            )
            emit_bias_cols()

            def emit_fk_merge(item):
                # previous kernel's mirrored column parts: Fk += cadd on
                # the DVE (data long ready by now), then store.
                kprev, Fkp, caddp = item
                nc.vector.tensor_add(
                    Fkp[:, 1:NSLAB], Fkp[:, 1:NSLAB], caddp[:]
                )
                nc.sync.dma_start(
                    out=f_out[:, kprev : kprev + 1].rearrange(
                        "(s p) o -> p (s o)", p=128
                    ),
                    in_=Fkp[:],
                )

            pending_fk = None
            for k in range(KPC):
                base = k * KERNEL_DIM
                Vp, S = Vk[k], Sk[k]
                Fk = fk_pool.tile([128, NSLAB], f32)
                # Column sums of E accumulated across slabs: cs[0, c] =
                # sum over slabs s with s*128+128 <= c of sum_b E_s[b, c]
                cs = cs_psum.tile([1, BATCH], f32)
                pending_cs = None  # (slab, E) deferred one slab

                def emit_cs(item, stop, cs=cs):
                    slab, E = item
                    c0 = slab * 128
                    start = c0 + 128
                    while start < BATCH:
                        # keep each matmul within one PSUM bank (512 f32)
                        fd = min(512 - (start % 512), BATCH - start)
                        nc.tensor.matmul(
                            cs[0:1, start : start + fd],
                            lhsT=ones_sb[:],
                            rhs=E[:, start - c0 : start - c0 + fd],
                            start=(slab == 0),
                            stop=(stop and start + fd >= BATCH),
                        )
                        start += fd

                for slab in range(NSLAB):
                    c0 = slab * 128
                    span = BATCH - c0
                    # PSUM accumulates P = R + bf16(-rs[c]/2); then
                    # exp(-2P + bias[b]) with bias[b] = bf16(-rs[b]) is
                    # exp(-sum_m |act_b - act_c|), exactly 1 on the diag.
                    D = d_psum.tile([128, BATCH], f32)
                    for off, fd in mm_chunks(span):
                        for G in range(2):
                            cls = 0 if (G == 0 and slab < 4) else 1
                            col = (k * 4 + G * 2 + cls) * 64
                            nc.tensor.matmul(
                                D[G * 64 : (G + 1) * 64, off : off + fd],
                                lhsT=selk_sb[:, col : col + 64],
                                rhs=rsh[:, c0 + off : c0 + off + fd],
                                start=True,
                                stop=False,
                            )
                    if pending_cs is not None:
                        emit_cs(pending_cs, stop=False)
                        pending_cs = None
                    if slab == 1 and pending_fk is not None:
                        emit_fk_merge(pending_fk)
                        pending_fk = None
                    if k + 1 < KPC:
                        # next kernel's inputs paced one plane per slab so
                        # the gather/broadcast DMA load never bunches up
                        # against latency-critical transfers
                        emit_inputs(
                            k + 1, nc.sync, [nc.gpsimd],
                            js=[GATHER_ORDER[slab]],
                        )
                    for j in J_ORDER:
                        if k not in Snhk and j in ACT_JS:
                            # k0 only: S_nh emitted mid-slab0 so the
                            # first relu slots aren't FIFO-blocked
                            emit_snh(k)
                        if k in Snhk:
                            S_nh, snh_idx = Snhk[k]
                        Tg = []
                        for G in range(2):
                            scol = slab * 2 + G
                            T = t_pool.tile([128, BATCH], bf16)
                            if act_slot(j, G, slab):
                                nc.scalar.activation(
                                    out=T[:, :span],
                                    in_=Vp[j][:, c0:BATCH],
                                    func=Act.Abs,
                                    scale=0.5,
                                    bias=S_nh[:, snh_idx[j], scol : scol + 1],
                                )
                            else:
                                nc.vector.tensor_scalar(
                                    out=T[:, :span],
                                    in0=Vp[j][:, c0:BATCH],
                                    scalar1=S[:, j, scol : scol + 1],
                                    scalar2=0.0,
                                    op0=Alu.subtract,
                                    op1=Alu.max,
                                )
                            Tg.append(T)
                        for off, fd in mm_chunks(span):
                            for G in range(2):
                                nc.tensor.matmul(
                                    D[G * 64 : (G + 1) * 64, off : off + fd],
                                    lhsT=onehot_sb[:],
                                    rhs=Tg[G][:, off : off + fd],
                                    start=False,
                                    stop=(j == J_ORDER[-1]),
                                )
                    # E = exp(-D) over the slab's c-window; accum_out gives
                    # the row part sum_c E directly.
                    E = e_pool.tile([128, BATCH], bf16)
                    nc.scalar.activation(
                        out=E[:, :span],
                        in_=D[:, :span],
                        func=Act.Exp,
                        scale=-2.0,
                        bias=bias_cols[:, slab * KPC + k : slab * KPC + k + 1],
                        accum_out=Fk[:, slab : slab + 1],
                    )
                    if span > 128:
                        pending_cs = (slab, E)
                if k + 1 < KPC:
                    emit_snh(k + 1)
                if pending_cs is not None:
                    emit_cs(pending_cs, stop=True)
                    pending_cs = None

                # cs row -> SBUF -> DRAM, gather as (p, t); the Fk merge is
                # deferred into the next kernel's DVE stream so the DVE
                # FIFO never blocks at the kernel boundary.
                cs_row = small_pool.tile([1, BATCH - 128], f32, tag="csr")
                nc.scalar.copy(cs_row[:], cs[0:1, 128:BATCH])
                cs_dram = dram_pool.tile([1, BATCH - 128], f32, name=f"csd{k}")
                nc.sync.dma_start(out=cs_dram[:], in_=cs_row[:])
                cadd = small_pool.tile([128, NSLAB - 1], f32)
                cd0 = cs_dram[0:1, 0:1]
                nc.sync.dma_start(
                    out=cadd[:],
                    in_=bass.AP(
                        tensor=cd0.tensor,
                        offset=cd0.offset,
                        ap=[[1, 128], [128, NSLAB - 1]],
                    ),
                )
                pending_fk = (k, Fk, cadd)
            emit_fk_merge(pending_fk)

    nc.compile()
    return nc


def _get_program():
    if "nc" not in _CACHE:
        _CACHE["nc"] = _build_program()
    return _CACHE["nc"]


def _prep_in_maps(x, W):
    bf16 = ml_dtypes.bfloat16
    xT = np.ascontiguousarray(x.T).astype(bf16)  # (D, B)
    onehot = (np.arange(128)[:, None] % 64 == np.arange(64)[None, :]).astype(bf16)
    rows = np.arange(KPC * KERNEL_DIM)
    blockdiag = np.zeros((KPC * KERNEL_DIM, NVAR * KPC), dtype=bf16)
    for v in range(NVAR):
        blockdiag[:, v * KPC : (v + 1) * KPC] = (
            rows[:, None] // KERNEL_DIM == np.arange(KPC)[None, :]
        ) & ((rows[:, None] % KERNEL_DIM) < RELU_M[v])
    selk = np.zeros((NVAR * KPC, KPC * 256), dtype=bf16)
    for k in range(KPC):
        for G in range(2):
            for cls in range(2):
                v = cls if G == 0 else 2
                col = (k * 4 + G * 2 + cls) * 64
                selk[v * KPC + k, col : col + 64] = 1
    in_maps = []
    for c in range(N_CORES):
        Wc = W[c * KPC : (c + 1) * KPC]  # (KPC, D, M)
        wTc = np.ascontiguousarray(
            Wc.transpose(1, 0, 2).reshape(INPUT_DIM, KPC * KERNEL_DIM)
        )
        in_maps.append(
            {
                "xT": xT,
                "wT": wTc.astype(bf16),
                "onehot": onehot,
                "blockdiag": blockdiag,
                "selk": selk,
            }
        )
    return in_maps


def run_hw(x, W, trace=False, **kwargs):
    from concourse.bass_utils import run_bass_kernel_spmd

    nc = _get_program()
    in_maps = _prep_in_maps(x, W)
    res = run_bass_kernel_spmd(
        nc, in_maps, core_ids=list(range(N_CORES)), trace=trace, **kwargs
    )
    F = np.concatenate([res.results[c]["f_out"] for c in range(N_CORES)], axis=1)
    return F.astype(np.float32), res


def kernel(x, W):
    x = np.asarray(x, dtype=np.float32)
    W = np.asarray(W, dtype=np.float32)
    F, _ = run_hw(x, W, trace=False)
    return np.concatenate([x, F], axis=1)


if __name__ == "__main__":
    x = np.random.randn(BATCH, INPUT_DIM).astype(np.float32)
    W = (
        np.random.randn(NB_KERNELS, INPUT_DIM, KERNEL_DIM)
        / np.sqrt(INPUT_DIM + KERNEL_DIM)
    ).astype(np.float32)
    out = kernel(x, W)
    print(out.shape, out.dtype)
